# revision 3
# baseline (speedup 1.0000x reference)
"""Trainium2 Bass kernel for nn_MoE_68839735821022 (moe_routing).

Expert-parallel MoE, no collectives, host-side routing/dispatch/combine.

Host: bit-exact routing (CPU jax), capacity positions, then a compact
slot layout: each core's 16 experts are sorted by descending occupancy
and padded to per-POSITION sizes n_j = max over cores of the j-th
sorted count.  The position sizes are baked into the (SPMD-uniform)
program; all per-core differences live in the data.  Softmax scores
are folded into the dispatched x columns on host (s*relu(W1x+b1) =
relu(s*W1x + s*b1) since s>0), so pad slots produce exactly zero.

Device (one SPMD program on 8 cores, no inter-core traffic):
  A:  hT[:, slot] = relu(sum_dt W1Tdt @ dispTdt + b1 (x) s)   (rank-1
      K=1 matmul adds the s-scaled bias), own 16 positions, emitted in
      4 quads so compute starts as soon as the first quad lands.
  B:  per slot m-chunk: y[:m, hid] = hT_chunk^T @ W2own -> psum ->
      bf16 -> ybuf[V, 4096] rows (exact m rows, 8KB contiguous each).
      W2 for the core's OWN experts over the FULL hidden dim streams
      through SBUF (16 x 1MB DMAs on the sync ring).
  diag (2D-sharded: 4 token groups x 2 hidden halves): per 128-token
      tile/half: psum = sum_dt zT @ wpT + ST @ b2 -> out[1024, 2048].
      Inputs ride the scalar HWDGE ring, parallel to the sync ring.
Host combine: out = assembled diag + bp + sum_k valid_k *
      ybuf_owner[row(n,k)]  (score already folded into ybuf rows).
"""

import os
import sys

import numpy as np

sys.path.insert(0, "/opt/trn_rl_repo")

# Problem constants (hardcoded per the harness contract).
DIM, HID, E, K, R, CAP = 1024, 4096, 128, 4, 128, 256
BS, SEQ = 1, 4096
N = BS * SEQ
NCORES = 8
EPC = E // NCORES          # experts per core
TGRP, HGRP = 4, 2          # diag GEMM sharding: token groups x hid halves
TPG = N // TGRP            # tokens per group (1024)
HH = HID // HGRP           # hidden half (2048)
NTT = TPG // 128           # token tiles per core for diag (8)
NQ = 4                     # A-input load quads
JPQ = EPC // NQ            # positions per quad

_CACHE = {}


def _bf16():
    import ml_dtypes
    return np.dtype(ml_dtypes.bfloat16)


def _routing_host(x, Wr, br):
    """Bit-exact replication of the reference's routing, on CPU jax."""
    import jax
    import jax.numpy as jnp

    cpu = jax.devices("cpu")[0]
    with jax.default_device(cpu):
        xf = jnp.asarray(np.asarray(x).reshape(-1, DIM))
        logits = xf @ jnp.asarray(np.asarray(Wr)).T + jnp.asarray(np.asarray(br))
        thr = jnp.quantile(jnp.abs(logits), 0.8)
        logits = jnp.where(jnp.abs(logits) < thr, 0.0, logits)
        topv, topi = jax.lax.top_k(logits, K)
        scores = jax.nn.softmax(topv, axis=-1)
        topi = np.asarray(topi)
        scores = np.asarray(scores)
    return topi, scores


def _positions(e_flat):
    """Reference capacity positions: running count per expert in flat order."""
    pos = np.empty(e_flat.shape[0], dtype=np.int64)
    counts = np.zeros(E, dtype=np.int64)
    for m, e in enumerate(e_flat):
        pos[m] = counts[e]
        counts[e] += 1
    return pos, counts


def _plan(counts):
    """Uniform (SPMD-safe) compact layout from per-expert counts."""
    cap = np.minimum(counts, CAP)
    perm = np.zeros((NCORES, EPC), np.int64)   # expert id at (core, position)
    nsort = np.zeros((NCORES, EPC), np.int64)
    for c in range(NCORES):
        es = np.arange(c * EPC, (c + 1) * EPC)
        order = np.argsort(-cap[es], kind="stable")
        perm[c] = es[order]
        nsort[c] = cap[es[order]]
    n_j = nsort.max(axis=0)                    # baked position sizes
    n_j = np.maximum(n_j, 1)
    loff = np.concatenate([[0], np.cumsum(n_j)])
    V = int(loff[-1])                          # slots per core (ragged)
    return perm, nsort, n_j.astype(int), loff.astype(int), V


def _bchunks(n_j, loff):
    """B-phase m-chunk table: (j, mi, m, col0)."""
    ch = []
    for j in range(EPC):
        nj, mi = int(n_j[j]), 0
        while nj > 0:
            m = min(nj, 128)
            ch.append((j, mi, m, int(loff[j]) + mi * 128))
            nj -= m
            mi += 1
    return ch


def _prep_inputs(x, Wr, br, diag, Wp, bp, W1, b1, W2, b2):
    bf16 = _bf16()
    xf = np.asarray(x, np.float32).reshape(-1, DIM)
    topi, scores = _routing_host(x, Wr, br)

    e_flat = topi.reshape(-1)
    s_flat = scores.reshape(-1)
    tok = np.repeat(np.arange(N), K)
    pos, counts = _positions(e_flat)
    valid = pos < CAP
    p_cap = np.minimum(pos, CAP - 1)

    perm, nsort, n_j, loff, V = _plan(counts)

    # expert -> (core, position-within-core)
    core_of = np.zeros(E, np.int64)
    j_of = np.zeros(E, np.int64)
    for c in range(NCORES):
        for j in range(EPC):
            core_of[perm[c, j]] = c
            j_of[perm[c, j]] = j

    lo_of = np.array([int(loff[j]) for j in range(EPC)])
    owner = core_of[e_flat]                       # [M]
    lrow = lo_of[j_of[e_flat]] + p_cap            # row in owner's ybuf

    # S^T: score-scatter matrix [E, N] (b2-fold matmul lhsT source)
    ST = np.zeros((E, N), np.float32)
    np.add.at(ST, (e_flat[valid], tok[valid]), s_flat[valid])
    ST = ST.astype(bf16)

    # diag path: z = x * (S @ diag)
    eff = np.einsum("nk,nkd->nd", scores, np.asarray(diag, np.float32)[topi])
    z = (xf * eff).astype(np.float32)             # [N, DIM]

    W1 = np.asarray(W1, np.float32)
    W2 = np.asarray(W2, np.float32)
    Wp = np.asarray(Wp, np.float32)
    b1 = np.asarray(b1, np.float32)
    b2 = np.asarray(b2, np.float32)

    in_maps = []
    for c in range(NCORES):
        ct, chh = c // HGRP, c % HGRP
        hs = slice(chh * HH, (chh + 1) * HH)
        ts = slice(ct * TPG, (ct + 1) * TPG)

        dispT = np.zeros((128, 8, V), bf16)
        w1T = np.zeros((128, EPC, 8, 128), bf16)
        b1row = np.zeros((1, EPC * 128), bf16)
        srow = np.zeros((1, V), bf16)
        w2e = np.zeros((EPC, R, HID), bf16)
        for j in range(EPC):
            e = perm[c, j]
            na = int(nsort[c, j])
            sel = (e_flat == e) & valid
            if na:
                order = np.argsort(pos[sel], kind="stable")
                cols = xf[tok[sel]][order] * s_flat[sel][order][:, None]
                dispT[:, :, loff[j]:loff[j] + na] = (
                    cols.T.reshape(8, 128, na).transpose(1, 0, 2))
                srow[0, loff[j]:loff[j] + na] = s_flat[sel][order]
            w1T[:, j] = W1[e].T.reshape(8, 128, 128).transpose(1, 0, 2)
            b1row[0, j * 128:(j + 1) * 128] = b1[e]
            w2e[j] = W2[e].T
        # z slice pre-tiled: [128(dim-in-chunk), tile, dt, 128(tok)]
        zT = z[ts].T.reshape(8, 128, NTT, 128).transpose(1, 2, 0, 3)
        im = {
            "w1T": np.ascontiguousarray(w1T),
            "b1row": b1row,
            "srow": srow,
            "w2e": np.ascontiguousarray(w2e),
            "zt": np.ascontiguousarray(zT.astype(bf16)),
            "wpT": np.ascontiguousarray(
                Wp[hs].T.reshape(8, 128, HH).transpose(1, 0, 2).astype(bf16)),
            "st": np.ascontiguousarray(ST[:, ts]),
            "b2s": np.ascontiguousarray(b2[:, hs].astype(bf16)),
        }
        for q in range(NQ):
            lo, hi = int(loff[q * JPQ]), int(loff[(q + 1) * JPQ])
            im[f"dispT{q}"] = np.ascontiguousarray(dispT[:, :, lo:hi])
        in_maps.append(im)
    sig = (V, tuple(int(v) for v in n_j))
    comb = (owner.reshape(N, K), lrow.reshape(N, K), valid.reshape(N, K))
    return in_maps, sig, comb


def _build_nc(sig):
    import concourse.bacc as bacc
    import concourse.mybir as mybir
    from concourse import tile

    V, n_j = sig
    n_j = list(n_j)
    loff = [0]
    for v in n_j:
        loff.append(loff[-1] + v)
    bch = _bchunks(n_j, loff)
    NB = len(bch)

    mdt = mybir.dt
    f32 = mdt.float32
    bf = mdt.bfloat16
    Relu = mybir.ActivationFunctionType.Relu
    Copy = mybir.ActivationFunctionType.Copy

    nc = bacc.Bacc("TRN2", target_bir_lowering=False, debug=False,
                   num_devices=NCORES)

    dispTq = [
        nc.declare_dram_parameter(
            f"dispT{q}", [128, 8, loff[(q + 1) * JPQ] - loff[q * JPQ]], bf,
            isOutput=False)
        for q in range(NQ)
    ]
    w1T = nc.declare_dram_parameter("w1T", [128, EPC, 8, 128], bf, isOutput=False)
    b1row = nc.declare_dram_parameter("b1row", [1, EPC * 128], bf, isOutput=False)
    srow = nc.declare_dram_parameter("srow", [1, V], bf, isOutput=False)
    w2e = nc.declare_dram_parameter("w2e", [EPC, R, HID], bf, isOutput=False)
    zt = nc.declare_dram_parameter("zt", [128, NTT, 8, 128], bf, isOutput=False)
    wpT = nc.declare_dram_parameter("wpT", [128, 8, HH], bf, isOutput=False)
    st = nc.declare_dram_parameter("st", [128, TPG], bf, isOutput=False)
    b2s = nc.declare_dram_parameter("b2s", [128, HH], bf, isOutput=False)
    out = nc.declare_dram_parameter("out", [TPG, HH], bf, isOutput=True)
    ybuf = nc.declare_dram_parameter("ybuf", [V, HID], bf, isOutput=True)

    with (
        tile.TileContext(nc) as tc,
        tc.tile_pool(name="pRes", bufs=1) as pRes,
        tc.tile_pool(name="pW2", bufs=4) as pW2,
        tc.tile_pool(name="pY", bufs=3) as pY,
        tc.tile_pool(name="pO", bufs=2) as pO,
    ):
        # ---- sync ring: A inputs (quads) interleaved with first w2 tiles --
        b1_t = pRes.tile([1, EPC * 128], bf, tag="b1row")
        nc.sync.dma_start(b1_t[:], b1row[:])
        sr_t = pRes.tile([1, V], bf, tag="srow")
        nc.sync.dma_start(sr_t[:], srow[:])
        d_res = pRes.tile([128, 8, V], bf, tag="disp")
        w1_res = pRes.tile([128, EPC, 8, 128], bf, tag="w1")
        w2_tiles = {}
        for q in range(NQ):
            lo, hi = loff[q * JPQ], loff[(q + 1) * JPQ]
            nc.sync.dma_start(d_res[:, :, lo:hi], dispTq[q][:])
            nc.sync.dma_start(w1_res[:, q * JPQ:(q + 1) * JPQ],
                              w1T[:, q * JPQ:(q + 1) * JPQ])
            if q < 3:
                w2_t = pW2.tile([128, HID], bf, tag="w2", name=f"w2_{q}")
                nc.sync.dma_start(w2_t[:], w2e[q])
                w2_tiles[q] = w2_t

        # ---- scalar ring: diag inputs, parallel to the sync ring ---------
        zt_t = pRes.tile([128, NTT, 8, 128], bf, tag="zt")
        nc.scalar.dma_start(zt_t[:, :NTT // 2], zt[:, :NTT // 2])
        wp_t = pRes.tile([128, 8, HH], bf, tag="wpT")
        nc.scalar.dma_start(wp_t[:, :, :HH // 2], wpT[:, :, :HH // 2])
        st_t = pRes.tile([128, TPG], bf, tag="st")
        nc.scalar.dma_start(st_t[:], st[:])
        b2_t = pRes.tile([128, HH], bf, tag="b2s")
        nc.scalar.dma_start(b2_t[:], b2s[:])
        nc.scalar.dma_start(zt_t[:, NTT // 2:], zt[:, NTT // 2:])
        nc.scalar.dma_start(wp_t[:, :, HH // 2:], wpT[:, :, HH // 2:])

        hT = pRes.tile([128, V], bf, tag="hT")

        # ---------------- Phase A: hT = relu(s*W1x + s*b1) ----------------
        with tc.tile_pool(name="psA", bufs=2, space="PSUM") as psA:
            for j in range(EPC):
                nj = n_j[j]
                lo = loff[j]
                ps = psA.tile([128, 256], f32, tag="psA", name=f"psA_{j}")
                for dt in range(8):
                    nc.tensor.matmul(ps[:, :nj], w1_res[:, j, dt, :],
                                     d_res[:, dt, lo:lo + nj],
                                     start=(dt == 0), stop=False)
                nc.tensor.matmul(ps[:, :nj], b1_t[:, j * 128:(j + 1) * 128],
                                 sr_t[:, lo:lo + nj],
                                 start=False, stop=True)
                nc.scalar.activation(hT[:, lo:lo + nj], ps[:, :nj], Relu)

        # ---------------- B (own-expert y) + diag, interleaved ------------
        with (
            tc.tile_pool(name="psB", bufs=2, space="PSUM") as psB,
            tc.tile_pool(name="psD", bufs=2, space="PSUM") as psD,
        ):
            # diag halves, h0 pass then h1 pass (wp loads in halves)
            dhalves = [(t, 0) for t in range(NTT)] + [(t, 1) for t in range(NTT)]
            di = 0

            def emit_diag(di):
                t, h = dhalves[di]
                ps = psD.tile([128, 2, 512], f32, tag="psD", name=f"psD_{t}_{h}")
                for dt in range(8):
                    for q in range(2):
                        c0 = h * 1024 + q * 512
                        nc.tensor.matmul(ps[:, q, :], zt_t[:, t, dt, :],
                                         wp_t[:, dt, c0:c0 + 512],
                                         start=(dt == 0), stop=False)
                for q in range(2):
                    c0 = h * 1024 + q * 512
                    nc.tensor.matmul(ps[:, q, :],
                                     st_t[:, t * 128:(t + 1) * 128],
                                     b2_t[:, c0:c0 + 512],
                                     start=False, stop=True)
                o_t = pO.tile([128, 1024], bf, tag="o", name=f"o_{t}_{h}")
                if h == 0:
                    nc.scalar.activation(o_t[:], ps[:], Copy)
                else:
                    nc.vector.tensor_copy(o_t[:], ps[:])
                nc.gpsimd.dma_start(
                    out[t * 128:(t + 1) * 128, h * 1024:h * 1024 + 1024],
                    o_t[:])

            for bi, (j, mi, m, col0) in enumerate(bch):
                if mi == 0:
                    if j in w2_tiles:
                        w2_t = w2_tiles[j]
                    else:
                        w2_t = pW2.tile([128, HID], bf, tag="w2",
                                        name=f"w2_{j}")
                        nc.sync.dma_start(w2_t[:], w2e[j])
                y_t = pY.tile([128, HID], bf, tag="y", name=f"y_{bi}")
                even = (bi % 2 == 0)
                for hq in range(4):
                    ps = psB.tile([128, 2, 512], f32, tag="psB",
                                  name=f"psB_{bi}_{hq}")
                    for q in range(2):
                        c0 = hq * 1024 + q * 512
                        nc.tensor.matmul(ps[:m, q, :], hT[:, col0:col0 + m],
                                         w2_t[:, c0:c0 + 512],
                                         start=True, stop=True)
                    if even:
                        nc.vector.tensor_copy(
                            y_t[:m, hq * 1024:(hq + 1) * 1024], ps[:m])
                    else:
                        nc.scalar.activation(
                            y_t[:m, hq * 1024:(hq + 1) * 1024], ps[:m], Copy)
                weng = nc.gpsimd if even else nc.scalar
                weng.dma_start(ybuf[col0:col0 + m, :], y_t[:m, :])
                # ration diag halves so they fill w2-stream gaps and are
                # done a few chunks before the B tail
                while di < len(dhalves) and di * (NB - 4) < (bi + 1) * len(dhalves):
                    emit_diag(di)
                    di += 1
            while di < len(dhalves):
                emit_diag(di)
                di += 1
    nc.compile()
    return nc


def _get_nc(sig):
    key = ("nc", sig)
    if key not in _CACHE:
        _CACHE[key] = _build_nc(sig)
    return _CACHE[key]


def kernel(x, Wr, br, diag, Wp, bp, W1, b1, W2, b2):
    import time

    from concourse.bass_utils import run_bass_kernel_spmd

    in_maps, sig, comb = _prep_inputs(x, Wr, br, diag, Wp, bp, W1, b1, W2, b2)
    nc = _get_nc(sig)
    trace = bool(int(os.environ.get("MOE_TRACE", "0")))
    res = None
    for attempt in range(3):
        try:
            res = run_bass_kernel_spmd(nc, in_maps, core_ids=list(range(NCORES)),
                                       trace=trace)
            break
        except Exception:
            # the axon terminal occasionally reports fewer cores transiently
            if attempt == 2:
                raise
            time.sleep(45)
    if trace:
        _CACHE["last_exec_time_ns"] = res.exec_time_ns
        _CACHE["last_results"] = res

    owner, lrow, valid = comb                  # [N, K] each
    V = sig[0]
    bp32 = np.asarray(bp, np.float32)

    acc = np.empty((N, HID), np.float32)
    for c in range(NCORES):
        ct, chh = c // HGRP, c % HGRP
        acc[ct * TPG:(ct + 1) * TPG, chh * HH:(chh + 1) * HH] = (
            res.results[c]["out"].astype(np.float32))
    acc += bp32[None, :]

    YB = np.concatenate([np.asarray(res.results[c]["ybuf"])
                         for c in range(NCORES)], axis=0)   # [8V, HID] bf16
    idx = owner * V + lrow                                   # [N, K]
    for k in range(K):
        ya = YB[idx[:, k]].astype(np.float32)
        ya *= valid[:, k].astype(np.float32)[:, None]
        acc += ya
    return acc.reshape(BS, SEQ, HID)
